# revision 21
# baseline (speedup 1.0000x reference)
"""Segmented (per-ray) exclusive cumprod of (1-alpha) -> transmittance, on 8 trn2 cores.

Strategy: rays are independent, so shard the packed sample buffer across
8 cores x 128 partitions = 1024 ray-aligned rows of ~8.2K samples each.
Each row's transmittance is an independent left-to-right recurrence

    T_t = flag_t ? 1 : oma_{t-1} * T_{t-1}

computed in a single pass by the vector engine's tensor_tensor_scan
(state = (data0 * state) max data1).  Inputs are sign-encoded into one
f32 stream w: w_t = -1 at segment starts, else oma_{t-1}; then
data0 = w (w*state <= 0 < 1 at flags, so the max picks the reset value)
and data1 = (w < 0) (1.0 at flags, 0 elsewhere, one DVE tensor_scalar).
Cost model: 27.2us/core vs a 23.8us DMA roofline (2 f32 passes over
~1.05M samples/core at ~358 GB/s); the DMA engines run gapless, the
remainder is fixed launch head/tail.
"""

import numpy as np

P = 128          # partitions per core
N_CORES = 8
N_CHUNKS = 3     # 3 double-buffered pipeline chunks is the perf sweet spot
_ALIGN = 4
# Cap per-chunk free dim so 3 pools x 3 bufs x [128, f_chunk] f32 stays well
# inside SBUF; ray distributions needing a larger F just get more chunks.
_F_CHUNK_MAX = 4096

_NC_CACHE: dict = {}


def _get_nc(F, chunks):
    key = (F, chunks)
    if key in _NC_CACHE:
        return _NC_CACHE[key]
    import concourse.tile as tile
    from concourse import bacc, mybir

    nc = bacc.Bacc("TRN2", num_devices=N_CORES)
    w_d = nc.dram_tensor("w", [P, F], mybir.dt.float32, kind="ExternalInput")
    t_d = nc.dram_tensor("t", [P, F], mybir.dt.float32, kind="ExternalOutput")
    bufs = min(len(chunks), 3)
    with tile.TileContext(nc) as tc:
        with (
            tc.tile_pool(name="w", bufs=bufs) as wpool,
            tc.tile_pool(name="m", bufs=bufs) as mpool,
            tc.tile_pool(name="o", bufs=bufs) as opool,
        ):
            t_prev = None
            off = 0
            for fc in chunks:
                sl = slice(off, off + fc)
                w_t = wpool.tile([P, fc], mybir.dt.float32)
                nc.sync.dma_start(w_t[:], w_d[:, sl])
                # DVE instructions have a single sync-wait slot, so keep the
                # whole chunk dataflow on DVE: the is_lt takes the DMA wait
                # and the scan then has only same-engine dependencies.
                d1 = mpool.tile([P, fc], mybir.dt.float32)
                nc.vector.tensor_scalar(
                    d1[:], w_t[:], 0.0, None, mybir.AluOpType.is_lt
                )
                t_t = opool.tile([P, fc], mybir.dt.float32)
                init = (
                    1.0
                    if t_prev is None
                    else t_prev[:, t_prev.shape[1] - 1 : t_prev.shape[1]]
                )
                nc.vector.tensor_tensor_scan(
                    t_t[:], w_t[:], d1[:], init,
                    mybir.AluOpType.mult, mybir.AluOpType.max,
                )
                nc.sync.dma_start(t_d[:, sl], t_t[:])
                t_prev = t_t
                off += fc
    nc.compile()
    _NC_CACHE[key] = nc
    return nc


def _row_partition(starts, n):
    """Split [0, n) into N_CORES*P rows, each starting at a ray boundary,
    minimizing the max row length (binary search + greedy packing)."""
    nrows = N_CORES * P
    edges = np.unique(np.concatenate([np.zeros(1, np.int64), starts]))
    edges = np.concatenate([edges, np.asarray([n], np.int64)])

    def pack(F):
        ends = np.empty(nrows, np.int64)
        start = 0
        for r in range(nrows):
            j = np.searchsorted(edges, start + F, side="right") - 1
            end = int(edges[j])
            if end <= start < n:
                return None  # a single ray longer than F
            start = min(end, n) if end > start else start
            ends[r] = start
        return ends if start >= n else None

    lo = max(int(np.diff(edges).max()), -(-n // nrows))
    hi = lo + n // nrows + 1
    while lo < hi:
        mid = (lo + hi) // 2
        if pack(mid) is not None:
            hi = mid
        else:
            lo = mid + 1
    ends = pack(lo)
    assert ends is not None
    row_starts = np.concatenate([np.zeros(1, np.int64), ends[:-1]])
    return row_starts, ends


_F_MAX = 32768   # beyond this the device program degenerates; use host fallback


def _plan(starts, n):
    row_starts, row_ends = _row_partition(starts, n)
    f_needed = max(int((row_ends - row_starts).max()), 1)
    n_chunks = max(N_CHUNKS, -(-f_needed // _F_CHUNK_MAX))
    f_chunk = -(-f_needed // n_chunks)
    f_chunk = -(-f_chunk // _ALIGN) * _ALIGN
    F = n_chunks * f_chunk
    if n_chunks == 3:
        # Slightly smaller last chunk: its scan (which gates the final
        # out-DMA) finishes right as the DMA engines drain the prior chunk,
        # keeping the DMA track gapless through the tail. Optimum ~0.3465*F
        # for the first two chunks per the cost-model sweep.
        x = (int(F * 0.3465) // _ALIGN) * _ALIGN
        chunks = (x, x, F - 2 * x)
    else:
        chunks = (f_chunk,) * n_chunks
    return row_starts, row_ends, F, chunks


def _host_transmittance(oma, starts, n):
    """Bounded-time fallback for degenerate ray layouts (e.g. one ray spanning
    most of the buffer) where the tiled device program would be pathological."""
    bounds = np.unique(np.concatenate([np.zeros(1, np.int64), starts]))
    T = np.empty(n, np.float32)
    for s, e in zip(bounds, np.concatenate([bounds[1:], [n]])):
        if e > s:
            T[s] = 1.0
            if e - s > 1:
                T[s + 1 : e] = np.cumprod(oma[s : e - 1], dtype=np.float32)
    return T


def kernel(alpha, ray_start_idx):
    alpha = np.asarray(alpha)
    starts = np.asarray(ray_start_idx).astype(np.int64)
    n = alpha.shape[0]
    oma = (np.float32(1.0) - np.asarray(alpha, np.float32).reshape(-1))

    # sign-encoded scan input
    w = np.empty(n, np.float32)
    w[1:] = oma[:-1]
    w[0] = -1.0
    w[starts] = -1.0

    row_starts, row_ends, F, chunks = _plan(starts, n)

    if F > _F_MAX:
        T = _host_transmittance(oma, starts, n)
        return _finish(T, oma, starts, n)

    w_packed = np.ones((N_CORES * P, F), np.float32)
    for k in range(N_CORES * P):
        s, e = row_starts[k], row_ends[k]
        w_packed[k, : e - s] = w[s:e]
    w_packed = w_packed.reshape(N_CORES, P, F)

    from concourse.bass_utils import run_bass_kernel_spmd

    nc = _get_nc(F, chunks)
    in_maps = [{"w": w_packed[c]} for c in range(N_CORES)]
    res = None
    for attempt in range(3):
        try:
            res = run_bass_kernel_spmd(nc, in_maps, list(range(N_CORES)))
            break
        except Exception:
            # rare transient NRT_EXEC_UNIT_UNRECOVERABLE seen under axon;
            # a re-execution of the same NEFF succeeds
            if attempt == 2:
                raise
            import time

            time.sleep(2.0 * (attempt + 1))
    t_rows = np.stack([res.results[c]["t"] for c in range(N_CORES)]).reshape(
        N_CORES * P, F
    )

    T = np.empty(n, np.float32)
    for k in range(N_CORES * P):
        s, e = row_starts[k], row_ends[k]
        T[s:e] = t_rows[k, : e - s]
    return _finish(T, oma, starts, n)


def _finish(T, oma, starts, n):
    ends = np.concatenate([starts[1:], np.asarray([n], np.int64)])
    idx = np.maximum(ends - 1, 0)
    bg = np.where(ends > starts, T[idx] * oma[idx], np.float32(1.0)).astype(
        np.float32
    )
    return T.reshape(-1, 1), bg
